# revision 2
# baseline (speedup 1.0000x reference)
"""Distributed Trainium2 kernel for nn_AlternateConvolution (node_layer branch).

Reference computation:
    d      = (H_e @ p.T)[:, 0]                    # [N_E] per-edge scalar
    G      = (T * d[None, :]) @ T.T               # [N_V, N_V]  (symmetric)
    M1     = eye + (1 - eye) * G                  # diagonal forced to 1
    A      = M1 * adj_v
    ret    = A @ (H_v @ weight) + bias            # [N_V, OUT_V]
    return (ret, H_e)

Distribution (8 NeuronCores, SPMD single NEFF):
    Row-shard the node dim: core c owns output rows R_c = [512c, 512(c+1)).
    It computes G.T[:, R_c] = (scaled T.T block) contracted against full T.T,
    masks with adj_v.T[:, R_c] (diagonal zeroed host-side), and accumulates
    ret[R_c] = A.T^T @ W2 on the fly.  The "diagonal forced to 1" term is
    applied as a rank-style correction ret[i] += adj_v[i,i] * W2[i] at the
    end, so no per-core control flow is needed (all 8 cores run one graph).

    All operand transposes/casts/tilings are done host-side so every device
    DMA is a contiguous 2D transfer; matmuls run fp16 with fp32 PSUM.
"""

import numpy as np

N_V, N_E = 4096, 16384
IN_V, OUT_V, IN_E = 128, 128, 64
NCORES = 8
RB = N_V // NCORES          # 512 output rows per core
ETB = 16                    # e superblocks (1024 edges each)
EL = 8                      # e chunks (128 edges) per superblock
E_T = ETB * EL              # 128 e chunks total
G_BLK = 8                   # j blocks of 512
JBL = 4                     # j chunks (128) per j block

_F16 = np.float16
_cache = {}


def _build():
    import concourse.mybir as mybir
    from concourse import bacc
    from concourse.tile import TileContext

    dt = mybir.dt
    f16, f32 = dt.float16, dt.float32

    nc = bacc.Bacc("TRN2", target_bir_lowering=False, debug=False,
                   num_devices=NCORES)

    # -------- DRAM parameters (host-pretiled layouts) --------
    # TTt[g, etb, p, el*512+j] = T[g*512+j, etb*1024+el*128+p]   (replicated)
    TTt = nc.dram_tensor("TTt", [G_BLK, ETB, 128, EL * 512], f16,
                         kind="ExternalInput")
    # TTB[etb, p, el*512+i] = T[c*512+i, etb*1024+el*128+p]      (per-core)
    TTB = nc.dram_tensor("TTB", [ETB, 128, EL * 512], f16, kind="ExternalInput")
    # ADJt[g, p, jbl*512+i] = adj_v[c*512+i, g*512+jbl*128+p], diag zeroed
    ADJt = nc.dram_tensor("ADJt", [G_BLK, 128, JBL * 512], f16,
                          kind="ExternalInput")
    HVT = nc.dram_tensor("HVT", [IN_V, N_V], f16, kind="ExternalInput")
    HVTB = nc.dram_tensor("HVTB", [IN_V, RB], f16, kind="ExternalInput")
    WT = nc.dram_tensor("WT", [IN_V, OUT_V], f16, kind="ExternalInput")
    HET = nc.dram_tensor("HET", [IN_E, N_E], f16, kind="ExternalInput")
    PC = nc.dram_tensor("PC", [IN_E, 1], f16, kind="ExternalInput")
    BIA = nc.dram_tensor("BIA", [128, OUT_V], f32, kind="ExternalInput")
    ADIA = nc.dram_tensor("ADIA", [128, RB // 128], f32, kind="ExternalInput")
    OUT = nc.dram_tensor("out", [RB, OUT_V], f32, kind="ExternalOutput")

    with TileContext(nc) as tc:
        const = tc.alloc_tile_pool(name="const", bufs=1)
        spool = tc.alloc_tile_pool(name="spool", bufs=1)
        hetp = tc.alloc_tile_pool(name="hetp", bufs=1)
        stage = tc.alloc_tile_pool(name="stage", bufs=2)
        psum0 = tc.alloc_tile_pool(name="psum0", bufs=2, space="PSUM")

        pc_sb = const.tile([IN_E, 1], f16)
        wt_sb = const.tile([IN_V, OUT_V], f16)
        hvt_sb = const.tile([IN_V, N_V], f16)
        hvtb_sb = const.tile([IN_V, RB], f16)
        bia_sb = const.tile([128, OUT_V], f32)
        adia_sb = const.tile([128, RB // 128], f32)
        w2_sb = const.tile([128, N_V], f16)          # H_v @ weight, [j, o] tiles
        w2b_sb = const.tile([128, RB], f32)          # (H_v @ weight)[R_c] tiles
        d_sb = const.tile([128, E_T], f32)           # per-edge scalars
        s_sb = spool.tile([128, E_T * 512], f16)     # scaled T.T[:, R_c]
        het_sb = hetp.tile([IN_E, N_E], f16)

        nc.sync.dma_start(out=pc_sb[:], in_=PC[:])
        nc.sync.dma_start(out=wt_sb[:], in_=WT[:])
        nc.sync.dma_start(out=hvt_sb[:], in_=HVT[:])
        nc.sync.dma_start(out=hvtb_sb[:], in_=HVTB[:])
        nc.sync.dma_start(out=bia_sb[:], in_=BIA[:])
        nc.sync.dma_start(out=adia_sb[:], in_=ADIA[:])
        nc.sync.dma_start(out=het_sb[:], in_=HET[:])

        # d[e] = (H_e @ p.T): one column of PSUM per 128-edge chunk
        d_ps = psum0.tile([128, E_T], f32, tag="dps")
        for et in range(E_T):
            nc.tensor.matmul(d_ps[:, et:et + 1],
                             het_sb[:, et * 128:(et + 1) * 128], pc_sb[:],
                             start=True, stop=True)
        nc.vector.tensor_copy(d_sb[:], d_ps[:])

        # W2 = H_v @ weight, laid out as [j-partition, o] tiles
        for jb in range(N_V // 128):
            w_ps = psum0.tile([128, OUT_V], f32, tag="wps")
            nc.tensor.matmul(w_ps[:], hvt_sb[:, jb * 128:(jb + 1) * 128],
                             wt_sb[:], start=True, stop=True)
            nc.vector.tensor_copy(w2_sb[:, jb * 128:(jb + 1) * 128], w_ps[:])
        for ic in range(RB // 128):
            wb_ps = psum0.tile([128, OUT_V], f32, tag="wps")
            nc.tensor.matmul(wb_ps[:], hvtb_sb[:, ic * 128:(ic + 1) * 128],
                             wt_sb[:], start=True, stop=True)
            nc.vector.tensor_copy(w2b_sb[:, ic * 128:(ic + 1) * 128], wb_ps[:])

        # S = T.T[:, R_c] * d[e]  (per-partition scale), resident in SBUF
        for etb in range(ETB):
            sraw = stage.tile([128, EL * 512], f16, tag="sraw")
            nc.sync.dma_start(out=sraw[:], in_=TTB[etb])
            for el in range(EL):
                et = etb * EL + el
                nc.vector.tensor_scalar_mul(
                    s_sb[:, et * 512:(et + 1) * 512],
                    sraw[:, el * 512:(el + 1) * 512],
                    d_sb[:, et:et + 1])

        psum0.release()
        stage.release()
        hetp.release()

        ttp = tc.alloc_tile_pool(name="ttp", bufs=3)
        adjp = tc.alloc_tile_pool(name="adjp", bufs=2)
        atp = tc.alloc_tile_pool(name="atp", bufs=6)
        outp = tc.alloc_tile_pool(name="outp", bufs=2)
        psum_m = tc.alloc_tile_pool(name="psum_m", bufs=4, space="PSUM")
        psum_r = tc.alloc_tile_pool(name="psum_r", bufs=1, space="PSUM")

        ret_ps = [psum_r.tile([128, OUT_V], f32, tag=f"ret{ic}",
                              name=f"ret{ic}") for ic in range(RB // 128)]

        # Main loop: G.T[jblock, R_c] = sum_e TT[e, j] * S[e, i]
        for g in range(G_BLK):
            adj_g = adjp.tile([128, JBL * 512], f16, tag="adjg", name="adj_g")
            nc.sync.dma_start(out=adj_g[:], in_=ADJt[g])
            m_ps = [psum_m.tile([128, 512], f32, tag="mps", name=f"m_{g}_{j}")
                    for j in range(JBL)]
            for etb in range(ETB):
                tt = ttp.tile([128, EL * 512], f16, tag="tt", name=f"tt_{g}_{etb}")
                nc.sync.dma_start(out=tt[:], in_=TTt[g, etb])
                for el in range(EL):
                    et = etb * EL + el
                    s_slice = s_sb[:, et * 512:(et + 1) * 512]
                    for jbl in range(JBL):
                        nc.tensor.matmul(
                            m_ps[jbl][:],
                            tt[:, el * 512 + jbl * 128:el * 512 + (jbl + 1) * 128],
                            s_slice,
                            start=(et == 0), stop=(et == E_T - 1))
            for jbl in range(JBL):
                jb = g * JBL + jbl
                at = atp.tile([128, 512], f16, tag="at", name=f"at_{jb}")
                nc.vector.tensor_mul(at[:], m_ps[jbl][:],
                                     adj_g[:, jbl * 512:(jbl + 1) * 512])
                for ic in range(RB // 128):
                    nc.tensor.matmul(ret_ps[ic][:],
                                     at[:, ic * 128:(ic + 1) * 128],
                                     w2_sb[:, jb * 128:(jb + 1) * 128],
                                     start=(jb == 0), stop=(jb == N_V // 128 - 1))

        # Epilogue: ret += adj_diag * W2[R_c] + bias, then store
        for ic in range(RB // 128):
            tmp = outp.tile([128, OUT_V], f32, tag="tmp", name=f"tmp{ic}")
            nc.vector.tensor_scalar_mul(tmp[:],
                                        w2b_sb[:, ic * 128:(ic + 1) * 128],
                                        adia_sb[:, ic:ic + 1])
            res = outp.tile([128, OUT_V], f32, tag="res", name=f"res{ic}")
            nc.vector.tensor_add(res[:], ret_ps[ic][:], tmp[:])
            res2 = outp.tile([128, OUT_V], f32, tag="res2", name=f"res2{ic}")
            nc.vector.tensor_add(res2[:], res[:], bia_sb[:])
            nc.sync.dma_start(out=OUT[ic * 128:(ic + 1) * 128, :], in_=res2[:])

        psum_r.release()
        psum_m.release()
        outp.release()
        atp.release()
        adjp.release()
        ttp.release()
        spool.release()
        const.release()

    nc.compile()
    return nc


def _prep_inputs(H_v, H_e, adj_v, T, weight, p, bias):
    """Host-side shard/retile. Returns in_maps for 8 cores."""
    TT16 = np.ascontiguousarray(T.T).astype(_F16)          # [N_E, N_V]
    # [g, etb, p, el, j] tiling of TT16
    A = TT16.reshape(ETB, EL, 128, G_BLK, 512)
    TTt = np.ascontiguousarray(A.transpose(3, 0, 2, 1, 4)).reshape(
        G_BLK, ETB, 128, EL * 512)

    HVT = np.ascontiguousarray(H_v.T).astype(_F16)         # [IN_V, N_V]
    WT = weight.astype(_F16)
    HET = np.ascontiguousarray(H_e.T).astype(_F16)         # [IN_E, N_E]
    PCm = np.ascontiguousarray(p.T).astype(_F16)           # [IN_E, 1]
    BIA = np.ascontiguousarray(
        np.broadcast_to(bias.astype(np.float32), (128, OUT_V)))
    adjT = adj_v.T                                          # [j, i] view
    diag = np.ascontiguousarray(np.diag(adj_v)).astype(np.float32)

    in_maps = []
    for c in range(NCORES):
        r0, r1 = c * RB, (c + 1) * RB
        TTb = TT16[:, r0:r1]                                # [N_E, RB]
        B = TTb.reshape(ETB, EL, 128, 512)
        TTB = np.ascontiguousarray(B.transpose(0, 2, 1, 3)).reshape(
            ETB, 128, EL * 512)
        adjblk = adjT[:, r0:r1].astype(_F16)                # [N_V, RB]
        adjblk[r0 + np.arange(RB), np.arange(RB)] = 0       # zero the diagonal
        C = adjblk.reshape(G_BLK, JBL, 128, 512)
        ADJt = np.ascontiguousarray(C.transpose(0, 2, 1, 3)).reshape(
            G_BLK, 128, JBL * 512)
        ADIA = np.ascontiguousarray(
            diag[r0:r1].reshape(RB // 128, 128).T)          # [128, RB//128]
        in_maps.append({
            "TTt": TTt,
            "TTB": TTB,
            "ADJt": ADJt,
            "HVT": HVT,
            "HVTB": np.ascontiguousarray(HVT[:, r0:r1]),
            "WT": WT,
            "HET": HET,
            "PC": PCm,
            "BIA": BIA,
            "ADIA": ADIA,
        })
    return in_maps


def kernel(H_v, H_e, adj_e, adj_v, T, weight, p, bias):
    from concourse.bass_utils import run_bass_kernel_spmd

    H_v = np.asarray(H_v, dtype=np.float32)
    H_e = np.asarray(H_e, dtype=np.float32)
    adj_v = np.asarray(adj_v, dtype=np.float32)
    T = np.asarray(T, dtype=np.float32)
    weight = np.asarray(weight, dtype=np.float32)
    p = np.asarray(p, dtype=np.float32)
    bias = np.asarray(bias, dtype=np.float32)

    if "nc" not in _cache:
        _cache["nc"] = _build()
    nc = _cache["nc"]

    in_maps = _prep_inputs(H_v, H_e, adj_v, T, weight, p, bias)
    res = run_bass_kernel_spmd(nc, in_maps, list(range(NCORES)))
    ret = np.concatenate([res.results[c]["out"] for c in range(NCORES)], axis=0)
    return (ret, H_e)


# revision 3
# speedup vs baseline: 1.3035x; 1.3035x over previous
"""Distributed Trainium2 kernel for nn_AlternateConvolution (node_layer branch).

Reference computation:
    d      = (H_e @ p.T)[:, 0]                    # [N_E] per-edge scalar
    G      = (T * d[None, :]) @ T.T               # [N_V, N_V]  (symmetric!)
    M1     = eye + (1 - eye) * G                  # diagonal forced to 1
    A      = M1 * adj_v
    ret    = A @ (H_v @ weight) + bias            # [N_V, OUT_V]
    return (ret, H_e)

Distribution (8 NeuronCores, SPMD single NEFF), v2 — exploits G symmetry:
    Core c owns output rows R_c = [512c, 512(c+1)) (the "i" side).  G is
    symmetric, so only node-block pairs at ring distance 0..4 need computing:
    core c computes G[j-block (c+g)%8, R_c] for g in {0..4} (the distance-4
    block is computed by both ends; 40 of 64 blocks total vs 64 in the naive
    scheme).  Blocks at distance 1..3 are exchanged via one AllGather; each
    core picks the three tiles it needs with a partition-id-derived dynamic
    DMA offset, DMA-transposes them (G[j,i] = G[i,j]), and folds them into
    its output GEMM.

    All per-core block indexing is pre-rotated host-side (TTt5 / ADJt / HVTR
    are stored in ring order (c+g)%8) so all 8 cores run one identical graph.
    The "diagonal forced to 1" term is applied as a correction
    ret[i] += adj_v[i,i] * W2[i] at the end (adj diagonal zeroed host-side).

    Operands are fp16 (host-cast); accumulation fp32 in PSUM.
"""

import numpy as np

N_V, N_E = 4096, 16384
IN_V, OUT_V, IN_E = 128, 128, 64
NCORES = 8
RB = N_V // NCORES          # 512 output rows per core
ETB = 16                    # e superblocks (1024 edges each)
EL = 8                      # e chunks (128 edges) per superblock
E_T = ETB * EL              # 128 e chunks total
G_BLK = 8                   # node blocks of 512
NG = 5                      # j blocks computed per core (ring distance 0..4)
JBL = 4                     # j chunks (128) per j block
GLIST = [1, 2, 3, 0, 4]     # exchange tiles first so the AllGather overlaps
TILE_BYTES = JBL * 128 * 512  # elements per exchanged [512, 512] tile

_F16 = np.float16
_cache = {}


def _build():
    import concourse.mybir as mybir
    from concourse import bacc
    from concourse.tile import TileContext

    dt = mybir.dt
    f16, f32 = dt.float16, dt.float32

    nc = bacc.Bacc("TRN2", target_bir_lowering=False, debug=False,
                   num_devices=NCORES)

    # -------- DRAM parameters (host-pretiled, per-core ring-rotated) -----
    # TTt5[g, etb, p, el*512+j] = T[b*512+j, etb*1024+el*128+p], b=(c+g)%8
    TTt5 = nc.dram_tensor("TTt5", [NG, ETB, 128, EL * 512], f16,
                          kind="ExternalInput")
    # TTB[etb, p, el*512+i] = T[c*512+i, etb*1024+el*128+p]
    TTB = nc.dram_tensor("TTB", [ETB, 128, EL * 512], f16, kind="ExternalInput")
    # ADJt[g, p, jbl*512+i] = adj_v[c*512+i, b*512+jbl*128+p], b=(c+g)%8,
    # diagonal zeroed (lives in g=0)
    ADJt = nc.dram_tensor("ADJt", [G_BLK, 128, JBL * 512], f16,
                          kind="ExternalInput")
    HVTR = nc.dram_tensor("HVTR", [IN_V, N_V], f16, kind="ExternalInput")
    HVTB = nc.dram_tensor("HVTB", [IN_V, RB], f16, kind="ExternalInput")
    WT = nc.dram_tensor("WT", [IN_V, OUT_V], f16, kind="ExternalInput")
    HET = nc.dram_tensor("HET", [IN_E, N_E], f16, kind="ExternalInput")
    PC = nc.dram_tensor("PC", [IN_E, 1], f16, kind="ExternalInput")
    BIA = nc.dram_tensor("BIA", [128, OUT_V], f32, kind="ExternalInput")
    ADIA = nc.dram_tensor("ADIA", [128, RB // 128], f32, kind="ExternalInput")
    OUT = nc.dram_tensor("out", [RB, OUT_V], f32, kind="ExternalOutput")

    with TileContext(nc) as tc:
        const = tc.alloc_tile_pool(name="const", bufs=1)
        spool = tc.alloc_tile_pool(name="spool", bufs=1)
        dramp = tc.alloc_tile_pool(name="dramp", bufs=1, space="DRAM")
        hetp = tc.alloc_tile_pool(name="hetp", bufs=1)
        stage = tc.alloc_tile_pool(name="stage", bufs=2)
        psum0 = tc.alloc_tile_pool(name="psum0", bufs=2, space="PSUM")

        pc_sb = const.tile([IN_E, 1], f16)
        wt_sb = const.tile([IN_V, OUT_V], f16)
        hvt_sb = const.tile([IN_V, N_V], f16)
        hvtb_sb = const.tile([IN_V, RB], f16)
        bia_sb = const.tile([128, OUT_V], f32)
        adia_sb = const.tile([128, RB // 128], f32)
        w2_sb = const.tile([128, N_V], f16)          # H_v @ weight (ring order)
        w2b_sb = const.tile([128, RB], f32)          # (H_v @ weight)[R_c]
        d_sb = const.tile([128, E_T], f32)           # per-edge scalars
        s_sb = spool.tile([128, E_T * 512], f16)     # scaled T.T[:, R_c]
        het_sb = hetp.tile([IN_E, N_E], f16)

        contrib = dramp.tile([3, JBL, 128, 512], f16, name="contrib")
        agout = dramp.tile([NCORES, 3 * TILE_BYTES], f16, name="agout",
                           addr_space="Shared")
        exst = dramp.tile([3, TILE_BYTES], f16, name="exst")

        nc.sync.dma_start(out=pc_sb[:], in_=PC[:])
        nc.sync.dma_start(out=wt_sb[:], in_=WT[:])
        nc.sync.dma_start(out=hvt_sb[:], in_=HVTR[:])
        nc.sync.dma_start(out=hvtb_sb[:], in_=HVTB[:])
        nc.sync.dma_start(out=bia_sb[:], in_=BIA[:])
        nc.sync.dma_start(out=adia_sb[:], in_=ADIA[:])
        nc.sync.dma_start(out=het_sb[:], in_=HET[:])

        # d[e] = (H_e @ p.T): one PSUM column per 128-edge chunk
        d_ps = psum0.tile([128, E_T], f32, tag="dps")
        for et in range(E_T):
            nc.tensor.matmul(d_ps[:, et:et + 1],
                             het_sb[:, et * 128:(et + 1) * 128], pc_sb[:],
                             start=True, stop=True)
        nc.vector.tensor_copy(d_sb[:], d_ps[:])

        # W2 = H_v @ weight in ring order ([j-partition, o] tiles)
        for jb in range(N_V // 128):
            w_ps = psum0.tile([128, OUT_V], f32, tag="wps")
            nc.tensor.matmul(w_ps[:], hvt_sb[:, jb * 128:(jb + 1) * 128],
                             wt_sb[:], start=True, stop=True)
            nc.vector.tensor_copy(w2_sb[:, jb * 128:(jb + 1) * 128], w_ps[:])
        for ic in range(RB // 128):
            wb_ps = psum0.tile([128, OUT_V], f32, tag="wps")
            nc.tensor.matmul(wb_ps[:], hvtb_sb[:, ic * 128:(ic + 1) * 128],
                             wt_sb[:], start=True, stop=True)
            nc.vector.tensor_copy(w2b_sb[:, ic * 128:(ic + 1) * 128], wb_ps[:])

        # S = T.T[:, R_c] * d[e]  (per-partition scale), resident in SBUF
        for etb in range(ETB):
            sraw = stage.tile([128, EL * 512], f16, tag="sraw")
            nc.sync.dma_start(out=sraw[:], in_=TTB[etb])
            for el in range(EL):
                et = etb * EL + el
                nc.vector.tensor_scalar_mul(
                    s_sb[:, et * 512:(et + 1) * 512],
                    sraw[:, el * 512:(el + 1) * 512],
                    d_sb[:, et:et + 1])

        psum0.release()
        stage.release()
        hetp.release()

        ttp = tc.alloc_tile_pool(name="ttp", bufs=3)
        adjp = tc.alloc_tile_pool(name="adjp", bufs=3)
        atp = tc.alloc_tile_pool(name="atp", bufs=6)
        gexp = tc.alloc_tile_pool(name="gexp", bufs=6)
        outp = tc.alloc_tile_pool(name="outp", bufs=2)
        psum_m = tc.alloc_tile_pool(name="psum_m", bufs=4, space="PSUM")
        psum_r = tc.alloc_tile_pool(name="psum_r", bufs=1, space="PSUM")

        ret_ps = [psum_r.tile([128, OUT_V], f32, tag=f"ret{ic}",
                              name=f"ret{ic}") for ic in range(RB // 128)]
        n_ret_groups = G_BLK * JBL  # 32 accumulation steps per ret bank
        ret_idx = 0

        # ---- main loop over computed j-blocks (ring order) ----
        for g in GLIST:
            adj_g = adjp.tile([128, JBL * 512], f16, tag="adjg",
                              name=f"adj_{g}")
            nc.sync.dma_start(out=adj_g[:], in_=ADJt[g])
            m_ps = [psum_m.tile([128, 512], f32, tag="mps", name=f"m_{g}_{j}")
                    for j in range(JBL)]
            for etb in range(ETB):
                tt = ttp.tile([128, EL * 512], f16, tag="tt",
                              name=f"tt_{g}_{etb}")
                nc.sync.dma_start(out=tt[:], in_=TTt5[g, etb])
                for el in range(EL):
                    et = etb * EL + el
                    s_slice = s_sb[:, et * 512:(et + 1) * 512]
                    for jbl in range(JBL):
                        nc.tensor.matmul(
                            m_ps[jbl][:],
                            tt[:, el * 512 + jbl * 128:el * 512 + (jbl + 1) * 128],
                            s_slice,
                            start=(et == 0), stop=(et == E_T - 1))
            for jbl in range(JBL):
                jb = g * JBL + jbl
                at = atp.tile([128, 512], f16, tag="at", name=f"at_{jb}")
                if g in (1, 2, 3):
                    # export raw G tile for the exchange, blend from the copy
                    gex = gexp.tile([128, 512], f16, tag="gex",
                                    name=f"gex_{jb}")
                    nc.vector.tensor_copy(gex[:], m_ps[jbl][:])
                    nc.sync.dma_start(out=contrib[g - 1, jbl], in_=gex[:])
                    nc.vector.tensor_mul(at[:], gex[:],
                                         adj_g[:, jbl * 512:(jbl + 1) * 512])
                else:
                    nc.vector.tensor_mul(at[:], m_ps[jbl][:],
                                         adj_g[:, jbl * 512:(jbl + 1) * 512])
                for ic in range(RB // 128):
                    nc.tensor.matmul(ret_ps[ic][:],
                                     at[:, ic * 128:(ic + 1) * 128],
                                     w2_sb[:, jb * 128:(jb + 1) * 128],
                                     start=(ret_idx == 0),
                                     stop=(ret_idx == n_ret_groups - 1))
                ret_idx += 1
            if g == 3:
                # all three exchange tiles staged -> gather across the chip
                nc.gpsimd.collective_compute(
                    "AllGather", mybir.AluOpType.bypass,
                    replica_groups=[list(range(NCORES))],
                    ins=[contrib[:].opt()],
                    outs=[agout[:].opt()])

        # ---- consume exchanged tiles: j-blocks at ring distance 5..7 ----
        # need G[j in block (c+d)%8, R_c] = transpose of core (c+d)%8's
        # distance-(8-d) tile, which sits at slot (7-d) of its contribution
        for d in (5, 6, 7):
            src = nc.gpsimd.alloc_register(f"src{d}")
            pid = nc.gpsimd.partition_id()
            nc.gpsimd.reg_add(src, pid, d)
            nc.gpsimd.reg_mod(src, src, NCORES)
            srcv = nc.gpsimd.snap(src, donate=True, min_val=0,
                                  max_val=NCORES - 1)
            nc.gpsimd.dma_start(
                out=exst[d - 5:d - 4, :],
                in_=agout[bass_ds(srcv, 1),
                          (7 - d) * TILE_BYTES:(8 - d) * TILE_BYTES])
        exr = exst[:].rearrange("a (r c) -> a r c", r=512)
        for d in (5, 6, 7):
            adj_g = adjp.tile([128, JBL * 512], f16, tag="adjg",
                              name=f"adj_{d}")
            nc.sync.dma_start(out=adj_g[:], in_=ADJt[d])
            for jbl in range(JBL):
                jb = d * JBL + jbl
                rt = gexp.tile([128, 512], f16, tag="gex", name=f"rt_{jb}")
                nc.sync.dma_start_transpose(
                    rt[:], exr[d - 5, :, jbl * 128:(jbl + 1) * 128])
                at = atp.tile([128, 512], f16, tag="at", name=f"at_{jb}")
                nc.vector.tensor_mul(at[:], rt[:],
                                     adj_g[:, jbl * 512:(jbl + 1) * 512])
                for ic in range(RB // 128):
                    nc.tensor.matmul(ret_ps[ic][:],
                                     at[:, ic * 128:(ic + 1) * 128],
                                     w2_sb[:, jb * 128:(jb + 1) * 128],
                                     start=(ret_idx == 0),
                                     stop=(ret_idx == n_ret_groups - 1))
                ret_idx += 1
        assert ret_idx == n_ret_groups

        # ---- epilogue: ret += adj_diag * W2[R_c] + bias, store ----
        for ic in range(RB // 128):
            tmp = outp.tile([128, OUT_V], f32, tag="tmp", name=f"tmp{ic}")
            nc.vector.tensor_scalar_mul(tmp[:],
                                        w2b_sb[:, ic * 128:(ic + 1) * 128],
                                        adia_sb[:, ic:ic + 1])
            res = outp.tile([128, OUT_V], f32, tag="res", name=f"res{ic}")
            nc.vector.tensor_add(res[:], ret_ps[ic][:], tmp[:])
            res2 = outp.tile([128, OUT_V], f32, tag="res2", name=f"res2{ic}")
            nc.vector.tensor_add(res2[:], res[:], bia_sb[:])
            nc.sync.dma_start(out=OUT[ic * 128:(ic + 1) * 128, :], in_=res2[:])

        psum_r.release()
        psum_m.release()
        outp.release()
        gexp.release()
        atp.release()
        adjp.release()
        ttp.release()
        dramp.release()
        spool.release()
        const.release()

    nc.compile()
    return nc


def bass_ds(start, size):
    import concourse.bass as bass
    return bass.ds(start, size)


def _prep_inputs(H_v, H_e, adj_v, T, weight, p, bias):
    """Host-side shard/retile/rotate. Returns in_maps for 8 cores."""
    TT16 = np.ascontiguousarray(T.T).astype(_F16)          # [N_E, N_V]
    A = TT16.reshape(ETB, EL, 128, G_BLK, 512)
    TTt_all = np.ascontiguousarray(A.transpose(3, 0, 2, 1, 4)).reshape(
        G_BLK, ETB, 128, EL * 512)                          # [b, etb, p, el*j]

    HVT = np.ascontiguousarray(H_v.T).astype(_F16)         # [IN_V, N_V]
    WT = weight.astype(_F16)
    HET = np.ascontiguousarray(H_e.T).astype(_F16)         # [IN_E, N_E]
    PCm = np.ascontiguousarray(p.T).astype(_F16)           # [IN_E, 1]
    BIA = np.ascontiguousarray(
        np.broadcast_to(bias.astype(np.float32), (128, OUT_V)))
    adjT = adj_v.T                                          # [j, i] view
    diag = np.ascontiguousarray(np.diag(adj_v)).astype(np.float32)

    in_maps = []
    for c in range(NCORES):
        r0, r1 = c * RB, (c + 1) * RB
        ring = [(c + g) % G_BLK for g in range(G_BLK)]
        TTb = TT16[:, r0:r1]                                # [N_E, RB]
        B = TTb.reshape(ETB, EL, 128, 512)
        TTB = np.ascontiguousarray(B.transpose(0, 2, 1, 3)).reshape(
            ETB, 128, EL * 512)
        adjblk = adjT[:, r0:r1].astype(_F16)                # [N_V, RB]
        adjblk[r0 + np.arange(RB), np.arange(RB)] = 0       # zero the diagonal
        C = adjblk.reshape(G_BLK, JBL, 128, 512).transpose(0, 2, 1, 3)
        ADJt = np.ascontiguousarray(C[ring]).reshape(G_BLK, 128, JBL * 512)
        # ring-rotated H_v.T columns (512-wide blocks)
        cols = np.concatenate([np.arange(b * RB, (b + 1) * RB) for b in ring])
        HVTR = np.ascontiguousarray(HVT[:, cols])
        ADIA = np.ascontiguousarray(
            diag[r0:r1].reshape(RB // 128, 128).T)          # [128, RB//128]
        in_maps.append({
            "TTt5": np.ascontiguousarray(TTt_all[ring[:NG]]),
            "TTB": TTB,
            "ADJt": ADJt,
            "HVTR": HVTR,
            "HVTB": np.ascontiguousarray(HVT[:, r0:r1]),
            "WT": WT,
            "HET": HET,
            "PC": PCm,
            "BIA": BIA,
            "ADIA": ADIA,
        })
    return in_maps


def kernel(H_v, H_e, adj_e, adj_v, T, weight, p, bias):
    from concourse.bass_utils import run_bass_kernel_spmd

    H_v = np.asarray(H_v, dtype=np.float32)
    H_e = np.asarray(H_e, dtype=np.float32)
    adj_v = np.asarray(adj_v, dtype=np.float32)
    T = np.asarray(T, dtype=np.float32)
    weight = np.asarray(weight, dtype=np.float32)
    p = np.asarray(p, dtype=np.float32)
    bias = np.asarray(bias, dtype=np.float32)

    if "nc" not in _cache:
        _cache["nc"] = _build()
    nc = _cache["nc"]

    in_maps = _prep_inputs(H_v, H_e, adj_v, T, weight, p, bias)
    res = run_bass_kernel_spmd(nc, in_maps, list(range(NCORES)))
    ret = np.concatenate([res.results[c]["out"] for c in range(NCORES)], axis=0)
    return (ret, H_e)


# revision 5
# speedup vs baseline: 1.3728x; 1.0532x over previous
"""Distributed Trainium2 kernel for nn_AlternateConvolution (node_layer branch).

Reference computation:
    d      = (H_e @ p.T)[:, 0]                    # [N_E] per-edge scalar
    G      = (T * d[None, :]) @ T.T               # [N_V, N_V]  (symmetric!)
    M1     = eye + (1 - eye) * G                  # diagonal forced to 1
    A      = M1 * adj_v
    ret    = A @ (H_v @ weight) + bias            # [N_V, OUT_V]
    return (ret, H_e)

Distribution (8 NeuronCores, SPMD single NEFF), v3 — balanced symmetry:
    Core c owns output rows R_c = [512c, 512(c+1)).  G is symmetric, so only
    node-block pairs at ring distance 0..4 are computed: core c computes
    G[j-block (c+g)%8, R_c] for g in {0..3} in full, and HALF of the
    distance-4 block — the two ends of each distance-4 pair compute disjoint
    halves of the e-contraction (a per-core host-side permutation of the
    e axis keeps the SPMD graph uniform) and exchange partials.  Every core
    thus does exactly 4.5 block-equivalents of matmul work (36/64 of naive).

    Blocks at distance 1..3 plus the distance-4 partial are exchanged via one
    AllGather; each core picks the tiles it needs with a partition-id-derived
    dynamic DMA offset, DMA-transposes them (G[j,i] = G[i,j] — also true for
    any e-subset partial sum), and folds them into its output GEMM.

    All per-core block indexing is pre-rotated host-side (TTt4/TTH/ADJt/HVTR
    are stored in ring order (c+g)%8) so all 8 cores run one identical graph.
    The "diagonal forced to 1" term is applied as a correction
    ret[i] += adj_v[i,i] * W2[i] at the end (adj diagonal zeroed host-side).

    Operands are fp16 (host-cast); accumulation fp32 in PSUM.
"""

import numpy as np

N_V, N_E = 4096, 16384
IN_V, OUT_V, IN_E = 128, 128, 64
NCORES = 8
RB = N_V // NCORES          # 512 output rows per core
ETB = 16                    # e superblocks (1024 edges each)
EL = 8                      # e chunks (128 edges) per superblock
E_T = ETB * EL              # 128 e chunks total
G_BLK = 8                   # node blocks of 512
JBL = 4                     # j chunks (128) per j block
TILE_ELEMS = JBL * 128 * 512  # elements per exchanged [512, 512] tile

_F16 = np.float16
_cache = {}


def _build():
    import concourse.mybir as mybir
    from concourse import bacc
    from concourse.tile import TileContext

    dt = mybir.dt
    f16, f32 = dt.float16, dt.float32

    nc = bacc.Bacc("TRN2", target_bir_lowering=False, debug=False,
                   num_devices=NCORES)

    # ------- DRAM parameters (host-pretiled, per-core ring-rotated) ------
    # TTt4[g, etb, p, el*512+j] = T[b*512+j, e(etb,el,p)], b=(c+g)%8, g=0..3
    TTt4 = nc.dram_tensor("TTt4", [4, ETB, 128, EL * 512], f16,
                          kind="ExternalInput")
    # THH: first half (per-core e-permutation) of the distance-4 block
    TTH = nc.dram_tensor("TTH", [ETB // 2, 128, EL * 512], f16,
                         kind="ExternalInput")
    # TTB[etb, p, el*512+i] = T[c*512+i, e(etb,el,p)]
    TTB = nc.dram_tensor("TTB", [ETB, 128, EL * 512], f16, kind="ExternalInput")
    # ADJt[g, p, jbl*512+i] = adj_v[c*512+i, b*512+jbl*128+p], b=(c+g)%8,
    # diagonal zeroed (lives in g=0)
    ADJt = nc.dram_tensor("ADJt", [G_BLK, 128, JBL * 512], f16,
                          kind="ExternalInput")
    HVTR = nc.dram_tensor("HVTR", [IN_V, N_V], f16, kind="ExternalInput")
    HVTB = nc.dram_tensor("HVTB", [IN_V, RB], f16, kind="ExternalInput")
    WT = nc.dram_tensor("WT", [IN_V, OUT_V], f16, kind="ExternalInput")
    HET = nc.dram_tensor("HET", [IN_E, N_E], f16, kind="ExternalInput")
    PC = nc.dram_tensor("PC", [IN_E, 1], f16, kind="ExternalInput")
    BIA = nc.dram_tensor("BIA", [128, OUT_V], f32, kind="ExternalInput")
    ADIA = nc.dram_tensor("ADIA", [128, RB // 128], f32, kind="ExternalInput")
    OUT = nc.dram_tensor("out", [RB, OUT_V], f32, kind="ExternalOutput")

    with TileContext(nc) as tc:
        const = tc.alloc_tile_pool(name="const", bufs=1)
        spool = tc.alloc_tile_pool(name="spool", bufs=1)
        dramp = tc.alloc_tile_pool(name="dramp", bufs=1, space="DRAM")
        hetp = tc.alloc_tile_pool(name="hetp", bufs=1)
        stage = tc.alloc_tile_pool(name="stage", bufs=2)
        psum0 = tc.alloc_tile_pool(name="psum0", bufs=2, space="PSUM")

        pc_sb = const.tile([IN_E, 1], f16)
        wt_sb = const.tile([IN_V, OUT_V], f16)
        hvt_sb = const.tile([IN_V, N_V], f16)
        hvtb_sb = const.tile([IN_V, RB], f16)
        bia_sb = const.tile([128, OUT_V], f32)
        adia_sb = const.tile([128, RB // 128], f32)
        w2_sb = const.tile([128, N_V], f16)          # H_v @ weight (ring order)
        w2b_sb = const.tile([128, RB], f32)          # (H_v @ weight)[R_c]
        d_sb = const.tile([128, E_T], f32)           # per-edge scalars
        s_sb = spool.tile([128, E_T * 512], f16)     # scaled T.T[:, R_c]
        het_sb = hetp.tile([IN_E, N_E], f16)

        contrib = dramp.tile([4, JBL, 128, 512], f16, name="contrib")
        agout = dramp.tile([NCORES, 4 * TILE_ELEMS], f16, name="agout",
                           addr_space="Shared")
        exst = dramp.tile([4, TILE_ELEMS], f16, name="exst")

        nc.sync.dma_start(out=pc_sb[:], in_=PC[:])
        nc.sync.dma_start(out=wt_sb[:], in_=WT[:])
        nc.sync.dma_start(out=hvt_sb[:], in_=HVTR[:])
        nc.sync.dma_start(out=hvtb_sb[:], in_=HVTB[:])
        nc.sync.dma_start(out=bia_sb[:], in_=BIA[:])
        nc.sync.dma_start(out=adia_sb[:], in_=ADIA[:])
        nc.sync.dma_start(out=het_sb[:], in_=HET[:])

        # d[e] = (H_e @ p.T): one PSUM column per 128-edge chunk; copy out
        # per-superblock so the S scaling (and the PE main loop behind it)
        # starts as early as possible
        d_ps = psum0.tile([128, E_T], f32, tag="dps")
        for etb in range(ETB):
            for el in range(EL):
                et = etb * EL + el
                nc.tensor.matmul(d_ps[:, et:et + 1],
                                 het_sb[:, et * 128:(et + 1) * 128], pc_sb[:],
                                 start=True, stop=True)
            nc.vector.tensor_copy(d_sb[:, etb * EL:(etb + 1) * EL],
                                  d_ps[:, etb * EL:(etb + 1) * EL])

        # W2 = H_v @ weight in ring order ([j-partition, o] tiles)
        for jb in range(N_V // 128):
            w_ps = psum0.tile([128, OUT_V], f32, tag="wps")
            nc.tensor.matmul(w_ps[:], hvt_sb[:, jb * 128:(jb + 1) * 128],
                             wt_sb[:], start=True, stop=True)
            nc.vector.tensor_copy(w2_sb[:, jb * 128:(jb + 1) * 128], w_ps[:])
        for ic in range(RB // 128):
            wb_ps = psum0.tile([128, OUT_V], f32, tag="wps")
            nc.tensor.matmul(wb_ps[:], hvtb_sb[:, ic * 128:(ic + 1) * 128],
                             wt_sb[:], start=True, stop=True)
            nc.vector.tensor_copy(w2b_sb[:, ic * 128:(ic + 1) * 128], wb_ps[:])

        # S = T.T[:, R_c] * d[e]  (per-partition scale), resident in SBUF
        for etb in range(ETB):
            sraw = stage.tile([128, EL * 512], f16, tag="sraw")
            nc.sync.dma_start(out=sraw[:], in_=TTB[etb])
            for el in range(EL):
                et = etb * EL + el
                nc.vector.tensor_scalar_mul(
                    s_sb[:, et * 512:(et + 1) * 512],
                    sraw[:, el * 512:(el + 1) * 512],
                    d_sb[:, et:et + 1])

        psum0.release()
        stage.release()
        hetp.release()

        ttp = tc.alloc_tile_pool(name="ttp", bufs=3)
        adjp = tc.alloc_tile_pool(name="adjp", bufs=3)
        atp = tc.alloc_tile_pool(name="atp", bufs=6)
        gexp = tc.alloc_tile_pool(name="gexp", bufs=6)
        g4p = tc.alloc_tile_pool(name="g4p", bufs=1)
        outp = tc.alloc_tile_pool(name="outp", bufs=2)
        psum_m = tc.alloc_tile_pool(name="psum_m", bufs=4, space="PSUM")
        psum_r = tc.alloc_tile_pool(name="psum_r", bufs=1, space="PSUM")

        ret_ps = [psum_r.tile([128, OUT_V], f32, tag=f"ret{ic}",
                              name=f"ret{ic}") for ic in range(RB // 128)]
        n_ret_groups = G_BLK * JBL  # 32 accumulation steps per ret bank
        ret_idx = 0

        def ret_mms(at, jb):
            nonlocal ret_idx
            for ic in range(RB // 128):
                nc.tensor.matmul(ret_ps[ic][:],
                                 at[:, ic * 128:(ic + 1) * 128],
                                 w2_sb[:, jb * 128:(jb + 1) * 128],
                                 start=(ret_idx == 0),
                                 stop=(ret_idx == n_ret_groups - 1))
            ret_idx += 1

        # own half of the distance-4 block, kept for the post-exchange sum
        gex4 = [g4p.tile([128, 512], f16, tag=f"g4_{j}", name=f"g4_{j}")
                for j in range(JBL)]

        # ---- main loop over computed j-blocks (ring order) ----
        # g = 1,2,3: full blocks, exported raw for the exchange
        # g = 4:     half e-range (the pair partner computes the other half)
        # g = 0:     full block, local only (contains the diagonal)
        for g in [1, 2, 3, 4, 0]:
            half = (g == 4)
            n_etb = ETB // 2 if half else ETB
            last_et = n_etb * EL - 1
            adj_g = None
            if not half:
                adj_g = adjp.tile([128, JBL * 512], f16, tag="adjg",
                                  name=f"adj_{g}")
                nc.sync.dma_start(out=adj_g[:], in_=ADJt[g])
            m_ps = [psum_m.tile([128, 512], f32, tag="mps", name=f"m_{g}_{j}")
                    for j in range(JBL)]
            for etb in range(n_etb):
                tt = ttp.tile([128, EL * 512], f16, tag="tt",
                              name=f"tt_{g}_{etb}")
                nc.sync.dma_start(out=tt[:],
                                  in_=TTH[etb] if half else TTt4[g, etb])
                for el in range(EL):
                    et = etb * EL + el
                    s_slice = s_sb[:, et * 512:(et + 1) * 512]
                    for jbl in range(JBL):
                        nc.tensor.matmul(
                            m_ps[jbl][:],
                            tt[:, el * 512 + jbl * 128:el * 512 + (jbl + 1) * 128],
                            s_slice,
                            start=(et == 0), stop=(et == last_et))
            for jbl in range(JBL):
                jb = g * JBL + jbl
                if half:
                    # stage the partial; blended later after the exchange
                    nc.vector.tensor_copy(gex4[jbl][:], m_ps[jbl][:])
                    nc.sync.dma_start(out=contrib[3, jbl], in_=gex4[jbl][:])
                    continue
                at = atp.tile([128, 512], f16, tag="at", name=f"at_{jb}")
                if g in (1, 2, 3):
                    gex = gexp.tile([128, 512], f16, tag="gex",
                                    name=f"gex_{jb}")
                    nc.vector.tensor_copy(gex[:], m_ps[jbl][:])
                    nc.sync.dma_start(out=contrib[g - 1, jbl], in_=gex[:])
                    nc.vector.tensor_mul(at[:], gex[:],
                                         adj_g[:, jbl * 512:(jbl + 1) * 512])
                else:
                    nc.vector.tensor_mul(at[:], m_ps[jbl][:],
                                         adj_g[:, jbl * 512:(jbl + 1) * 512])
                ret_mms(at, jb)
            if g == 4:
                # all four exchange tiles staged -> gather across the chip
                nc.gpsimd.collective_compute(
                    "AllGather", mybir.AluOpType.bypass,
                    replica_groups=[list(range(NCORES))],
                    ins=[contrib[:].opt()],
                    outs=[agout[:].opt()])

        # ---- consume exchanged tiles ----
        # distance d in 5..7: G[j in block (c+d)%8, R_c] = transpose of core
        # (c+d)%8's distance-(8-d) tile (slot 7-d of its contribution).
        # distance 4: the pair partner's half-partial (slot 3), transposed,
        # summed with our own half.
        for d in (4, 5, 6, 7):
            src = nc.gpsimd.alloc_register(f"src{d}")
            pid = nc.gpsimd.partition_id()
            nc.gpsimd.reg_add(src, pid, d)
            nc.gpsimd.reg_mod(src, src, NCORES)
            srcv = nc.gpsimd.snap(src, donate=True, min_val=0,
                                  max_val=NCORES - 1)
            slot = 3 if d == 4 else 7 - d
            nc.gpsimd.dma_start(
                out=exst[d - 4:d - 3, :],
                in_=agout[_ds(srcv, 1),
                          slot * TILE_ELEMS:(slot + 1) * TILE_ELEMS])
        exr = exst[:].rearrange("a (r c) -> a r c", r=512)
        for d in (4, 5, 6, 7):
            adj_g = adjp.tile([128, JBL * 512], f16, tag="adjg",
                              name=f"adj_{d}")
            nc.sync.dma_start(out=adj_g[:], in_=ADJt[d])
            for jbl in range(JBL):
                jb = d * JBL + jbl
                rt = gexp.tile([128, 512], f16, tag="gex", name=f"rt_{jb}")
                nc.sync.dma_start_transpose(
                    rt[:], exr[d - 4, :, jbl * 128:(jbl + 1) * 128])
                at = atp.tile([128, 512], f16, tag="at", name=f"at_{jb}")
                if d == 4:
                    ats = atp.tile([128, 512], f16, tag="at", name=f"ats_{jb}")
                    nc.vector.tensor_add(ats[:], rt[:], gex4[jbl][:])
                    nc.vector.tensor_mul(at[:], ats[:],
                                         adj_g[:, jbl * 512:(jbl + 1) * 512])
                else:
                    nc.vector.tensor_mul(at[:], rt[:],
                                         adj_g[:, jbl * 512:(jbl + 1) * 512])
                ret_mms(at, jb)
        assert ret_idx == n_ret_groups

        # ---- epilogue: ret += adj_diag * W2[R_c] + bias, store ----
        for ic in range(RB // 128):
            tmp = outp.tile([128, OUT_V], f32, tag="tmp", name=f"tmp{ic}")
            nc.vector.tensor_scalar_mul(tmp[:],
                                        w2b_sb[:, ic * 128:(ic + 1) * 128],
                                        adia_sb[:, ic:ic + 1])
            res = outp.tile([128, OUT_V], f32, tag="res", name=f"res{ic}")
            nc.vector.tensor_add(res[:], ret_ps[ic][:], tmp[:])
            res2 = outp.tile([128, OUT_V], f32, tag="res2", name=f"res2{ic}")
            nc.vector.tensor_add(res2[:], res[:], bia_sb[:])
            nc.sync.dma_start(out=OUT[ic * 128:(ic + 1) * 128, :], in_=res2[:])

        psum_r.release()
        psum_m.release()
        outp.release()
        g4p.release()
        gexp.release()
        atp.release()
        adjp.release()
        ttp.release()
        dramp.release()
        spool.release()
        const.release()

    nc.compile()
    return nc


def _ds(start, size):
    import concourse.bass as bass
    return bass.ds(start, size)


def _prep_inputs(H_v, H_e, adj_v, T, weight, p, bias):
    """Host-side shard/retile/rotate/e-permute. Returns in_maps for 8 cores."""
    TT16 = np.ascontiguousarray(T.T).astype(_F16)          # [N_E, N_V]
    A = TT16.reshape(ETB, EL, 128, G_BLK, 512)
    TTt_all = np.ascontiguousarray(A.transpose(3, 0, 2, 1, 4)).reshape(
        G_BLK, ETB, 128, EL * 512)                          # [b, etb, p, el*j]

    HVT = np.ascontiguousarray(H_v.T).astype(_F16)         # [IN_V, N_V]
    WT = weight.astype(_F16)
    HET_n = np.ascontiguousarray(H_e.T).astype(_F16)       # [IN_E, N_E]
    PCm = np.ascontiguousarray(p.T).astype(_F16)           # [IN_E, 1]
    BIA = np.ascontiguousarray(
        np.broadcast_to(bias.astype(np.float32), (128, OUT_V)))
    adjT = adj_v.T                                          # [j, i] view
    diag = np.ascontiguousarray(np.diag(adj_v)).astype(np.float32)

    in_maps = []
    for c in range(NCORES):
        r0, r1 = c * RB, (c + 1) * RB
        ring = [(c + g) % G_BLK for g in range(G_BLK)]
        # e-superblock permutation: upper cores swap e-halves so the two
        # ends of each distance-4 pair compute disjoint halves
        if c < NCORES // 2:
            eperm = list(range(ETB))
        else:
            eperm = list(range(ETB // 2, ETB)) + list(range(ETB // 2))
        TTb = TT16[:, r0:r1]                                # [N_E, RB]
        B = TTb.reshape(ETB, EL, 128, 512)
        TTB = np.ascontiguousarray(
            B.transpose(0, 2, 1, 3)[eperm]).reshape(ETB, 128, EL * 512)
        HET = np.ascontiguousarray(
            HET_n.reshape(IN_E, ETB, EL * 128)[:, eperm]).reshape(IN_E, N_E)
        adjblk = adjT[:, r0:r1].astype(_F16)                # [N_V, RB]
        adjblk[r0 + np.arange(RB), np.arange(RB)] = 0       # zero the diagonal
        C = adjblk.reshape(G_BLK, JBL, 128, 512).transpose(0, 2, 1, 3)
        ADJt = np.ascontiguousarray(C[ring]).reshape(G_BLK, 128, JBL * 512)
        cols = np.concatenate([np.arange(b * RB, (b + 1) * RB) for b in ring])
        HVTR = np.ascontiguousarray(HVT[:, cols])
        ADIA = np.ascontiguousarray(
            diag[r0:r1].reshape(RB // 128, 128).T)          # [128, RB//128]
        ttg = TTt_all[:, eperm]                             # e-permuted blocks
        in_maps.append({
            "TTt4": np.ascontiguousarray(ttg[ring[:4]]),
            "TTH": np.ascontiguousarray(ttg[ring[4], :ETB // 2]),
            "TTB": TTB,
            "ADJt": ADJt,
            "HVTR": HVTR,
            "HVTB": np.ascontiguousarray(HVT[:, r0:r1]),
            "WT": WT,
            "HET": HET,
            "PC": PCm,
            "BIA": BIA,
            "ADIA": ADIA,
        })
    return in_maps


def kernel(H_v, H_e, adj_e, adj_v, T, weight, p, bias):
    from concourse.bass_utils import run_bass_kernel_spmd

    H_v = np.asarray(H_v, dtype=np.float32)
    H_e = np.asarray(H_e, dtype=np.float32)
    adj_v = np.asarray(adj_v, dtype=np.float32)
    T = np.asarray(T, dtype=np.float32)
    weight = np.asarray(weight, dtype=np.float32)
    p = np.asarray(p, dtype=np.float32)
    bias = np.asarray(bias, dtype=np.float32)

    if "nc" not in _cache:
        _cache["nc"] = _build()
    nc = _cache["nc"]

    in_maps = _prep_inputs(H_v, H_e, adj_v, T, weight, p, bias)
    res = run_bass_kernel_spmd(nc, in_maps, list(range(NCORES)))
    ret = np.concatenate([res.results[c]["out"] for c in range(NCORES)], axis=0)
    return (ret, H_e)
